# revision 22
# baseline (speedup 1.0000x reference)
"""Trainium2 Bass kernel for nn_BitwiseMultipyLogis (gnn_message_passing).

Reference computation (L=8 layers, N=100000 nodes, F=128 features):
    proj    = tanh(node_features @ trans + bias)          # [L, N, F]
    bitwise = proj * proj[layer_predict]                  # [L, N, F]
    bitwise = einsum('lnf,lfg->lng', bitwise, theta)      # [L, N, F]
    scores  = sigmoid(bitwise @ logis_w[0] + logis_b)     # [L, N]
    weights = softmax(scores, axis=0)                     # [L, N]
    out     = proj[layer_predict] + sum_l weights[l]*proj[l]   # [N, F]

Algebraic simplification: theta only feeds the logis_w dot product, so
    scores[l,n] = sigmoid( sum_f proj[l,n,f]*proj[lp,n,f]*v[l,f] + logis_b )
with v[l] = theta[l] @ logis_w[0] precomputed on host.  This removes the
entire [L,N,F]x[L,F,F] einsum (half the FLOPs).

Device strategy (8 NeuronCores, data-parallel over N, 12500 nodes/core),
v3 — engine-balanced, instruction-count-minimized:
  * transposed layout [F=128 partitions, node columns]; host pre-packs
    node_features to [NT, 128, L, 512] bf16 so ONE DMA loads a whole tile.
  * projT = trans^T @ xT on TensorE (bf16), tanh+bias on ScalarE.
  * all 8 score rows accumulate into ONE [8, 512] PSUM tile via one-hot
    v columns; sigmoid+softmax with no table swap:
    exp(sigmoid(x)) = exp(0.5*tanh(x/2) + 0.5)  -> 2 activations per tile.
  * softmax denominators of a 4-tile group accumulate into one [97, 512]
    PSUM tile (rows 0/32/64/96) so ONE DVE reciprocal serves 4 tiles.
  * weights normalized as rows: w8 = e8 * bcast8(recip); the projlp term
    is folded into layer lp's y-multiply via scalar_tensor_tensor
    (y_lp = (wb_lp + 1) * proj_lp), so no final add is needed.
  * weight broadcast rows->128 partitions via [8,128] row-ones lhsT.
  * aggregation: 32 transpose-accumulate matmuls sum y tiles directly into
    a NODE-MAJOR bf16 PSUM tile [128, 4, 128]; ScalarE copies it to SBUF
    and 4 DMAs store [128 nodes, 128 f] blocks.  Output is node-major so
    the host does no transpose.
"""

import numpy as np
import ml_dtypes
from contextlib import ExitStack
from concurrent.futures import ThreadPoolExecutor

import concourse.bass as bass
import concourse.mybir as mybir
import concourse.tile as tile
from concourse import bacc
from concourse.bass import ts

BF16 = mybir.dt.bfloat16
F32 = mybir.dt.float32
AF = mybir.ActivationFunctionType
ALU = mybir.AluOpType

L, N, F = 8, 100000, 128
CORES = 8
NS = N // CORES            # 12500 nodes per core
TILE = 512                 # node columns per tile (one f32 PSUM bank)
NT = (NS + TILE - 1) // TILE   # 25
NSP = NT * TILE            # 12800 (padded)
GRP = 3                    # tiles per reciprocal group (rows 0/32/64)
NB = TILE // 128           # 128-node blocks per tile

BF = ml_dtypes.bfloat16


def _body(tc, out, ins, lp: int, logis_b: float, nt: int):
    nc = tc.nc
    with ExitStack() as ctx:
        const = ctx.enter_context(tc.tile_pool(name="const", bufs=1))
        xts = ctx.enter_context(tc.tile_pool(name="xts", bufs=7))
        projp = ctx.enter_context(tc.tile_pool(name="projp", bufs=3, space="PSUM"))
        projs = ctx.enter_context(tc.tile_pool(name="projs", bufs=8))
        bits = ctx.enter_context(tc.tile_pool(name="bits", bufs=18))
        scp = ctx.enter_context(tc.tile_pool(name="scp", bufs=1, space="PSUM"))
        scs = ctx.enter_context(tc.tile_pool(name="scs", bufs=2))
        e8s = ctx.enter_context(tc.tile_pool(name="e8s", bufs=8))
        sep = ctx.enter_context(tc.tile_pool(name="sep", bufs=1, space="PSUM"))
        rcs = ctx.enter_context(tc.tile_pool(name="rcs", bufs=3))
        w8s = ctx.enter_context(tc.tile_pool(name="w8s", bufs=3))
        wbp = ctx.enter_context(tc.tile_pool(name="wbp", bufs=2, space="PSUM"))
        ys = ctx.enter_context(tc.tile_pool(name="ys", bufs=18))
        aggp = ctx.enter_context(tc.tile_pool(name="aggp", bufs=1, space="PSUM"))
        outs = ctx.enter_context(tc.tile_pool(name="outs", bufs=2))

        trans_sb = const.tile([128, 128], BF16)
        nc.sync.dma_start(trans_sb[:], ins["trans"])
        # v8oh[:, k, l, :]: one-hot lhsT; column 32k+l holds v[l] so the
        # score row of layer l for group-slot k lands at PSUM partition 32k+l.
        v8oh_sb = const.tile([128, GRP, L, 72], BF16)
        nc.sync.dma_start(v8oh_sb[:], ins["v8oh"])
        # rowones[32k+l, l, :] all-ones: broadcasts weight row 32k+l across
        # all 128 output partitions (lhsT base partition 32k is legal).
        rowones_sb = const.tile([72, L, 128], BF16)
        nc.sync.dma_start(rowones_sb[:], ins["rowones"])
        # sumsel[32k:32k+8, k, :]: ones in columns 32k..32k+7 -> the layer
        # sum lands REPLICATED on partitions 32k..32k+7, so the normalize
        # multiply can read it without a separate broadcast matmul.
        sumsel_sb = const.tile([72, GRP, 72], BF16)
        nc.sync.dma_start(sumsel_sb[:], ins["sumsel"])
        ident_sb = const.tile([128, 128], BF16)
        nc.sync.dma_start(ident_sb[:], ins["ident"])
        bias_sb = const.tile([128, 1], F32)
        nc.sync.dma_start(bias_sb[:], ins["biasc"])
        lb_bias = const.tile([128, 1], F32)
        nc.gpsimd.memset(lb_bias[:], 0.5 * logis_b)
        half_bias = const.tile([128, 1], F32)
        nc.gpsimd.memset(half_bias[:], 0.5)
        zbias = const.tile([128, 1], F32)
        nc.gpsimd.memset(zbias[:], 0.0)

        xt = ins["xt"]
        xt_tiles = {}
        PF = 5  # DMA prefetch distance (tiles)

        def issue_in_dma(t):
            if t < nt:
                xt_sb = xts.tile([128, L, TILE], BF16, tag="xt")
                nc.sync.dma_start(xt_sb[:], xt[t])
                xt_tiles[t] = xt_sb

        proj_t, bits_t, sc_t, e8_t, y_t, wb_t = {}, {}, {}, {}, {}, {}
        seP_g, recb_g, w8_t = {}, {}, {}

        # Deep software pipeline with instruction-level interleave: iteration
        # i processes tile i's proj, tile i-1's bits, tile i-2's scores,
        # tile i-3's denominator, tile i-6's broadcasts/products and tile
        # i-7's aggregation.  Interleaving the matmul families keeps every
        # PE instruction's dependencies satisfied ~an iteration in advance,
        # so the PE streams at its back-to-back pitch instead of stalling
        # on fresh cross-engine semaphores.

        for t in range(min(PF, nt)):
            issue_in_dma(t)
        for i in range(nt + 8):
            issue_in_dma(i + PF)
            tA, tB, tC, tD, tE, tF = i, i - 1, i - 2, i - 3, i - 6, i - 7
            doA = tA < nt
            doB = 0 <= tB < nt
            doC = 0 <= tC < nt
            doD = 0 <= tD < nt
            doE = 0 <= tE < nt
            doF = 0 <= tF < nt

            if doA:
                proj = projs.tile([128, L, TILE], BF16, tag="proj")
                proj_t[tA] = proj
            if doB:
                bits_t[tB] = []
                plp2 = bits.tile([128, 2, TILE], BF16, tag="plp2",
                                 name="plp2", bufs=2)
                nc.vector.tensor_copy(plp2[:, 0, :], proj_t[tB][:, lp, :])
                nc.vector.tensor_copy(plp2[:, 1, :], proj_t[tB][:, lp, :])
                plp2_t = plp2
            if doC:
                sc_t[tC] = scp.tile([72, TILE], F32, tag="sc", name="sc")
            if doE:
                # normalized weight rows for tile tE (all SBUF bf16: 2x DVE)
                g, k = divmod(tE, GRP)
                q = 32 * k
                w8 = w8s.tile([72, TILE], BF16, tag="w8")
                nc.vector.tensor_mul(w8[q:q + 8, :], e8_t.pop(tE)[q:q + 8, :],
                                     recb_g[g][q:q + 8, :])
                w8_t[tE] = w8
                y_t[tE] = []
                wb_t[tE] = []

            for l in range(L):
                if doA:
                    pp = projp.tile([128, TILE], F32, tag="pp")
                    nc.tensor.matmul(pp[:], trans_sb[:],
                                     xt_tiles[tA][:, l, :],
                                     start=True, stop=True)
                    nc.scalar.activation(proj_t[tA][:, l, :], pp[:], AF.Tanh,
                                         bias=bias_sb[:, 0:1], scale=1.0)
                if doB:
                    if l in (0, 4):
                        # paired product on GpSimd: one op covers 2 layers
                        bit2 = bits.tile([128, 2, TILE], BF16, tag="bit2",
                                         name="bit2", bufs=4)
                        nc.gpsimd.tensor_mul(bit2[:], proj_t[tB][:, l:l + 2, :],
                                             plp2_t[:])
                        bits_t[tB].append(bit2[:, 0, :])
                        bits_t[tB].append(bit2[:, 1, :])
                    elif l in (2, 3, 6, 7):
                        bit = bits.tile([128, TILE], BF16, tag="bit")
                        nc.vector.tensor_mul(bit[:], proj_t[tB][:, l, :],
                                             proj_t[tB][:, lp, :])
                        bits_t[tB].append(bit[:])
                if doC:
                    kC = tC % GRP
                    nc.tensor.matmul(sc_t[tC][0:32 * kC + 8, :],
                                     v8oh_sb[:, kC, l, 0:32 * kC + 8],
                                     bits_t[tC][l],
                                     start=(l == 0), stop=(l == L - 1))
                if doE:
                    qE = 32 * (tE % GRP)
                    wb = wbp.tile([128, TILE], F32, tag="wb")
                    nc.tensor.matmul(wb[:], rowones_sb[qE:qE + 8, l, :],
                                     w8_t[tE][qE:qE + 8, :],
                                     start=True, stop=True)
                    wb_t[tE].append(wb)
                    if l >= 1:
                        _emit_y(nc, lp, tE, l - 1, wb_t, proj_t, y_t, ys)

            if doA:
                xt_tiles.pop(tA)
            if doC:
                del bits_t[tC]
                sc = sc_t.pop(tC)
                # e = exp(sigmoid(raw + lb)) with no table swap:
                # u = tanh(0.5*raw + 0.5*lb); e = exp(0.5*u + 0.5)
                kC = tC % GRP
                q = 32 * kC
                sct = scs.tile([72, TILE], F32, tag="sct")
                nc.scalar.activation(sct[q:q + 8, :], sc[q:q + 8, :], AF.Tanh,
                                     bias=lb_bias[q:q + 8, :], scale=0.5)
                e8 = e8s.tile([72, TILE], BF16, tag="e8")
                nc.scalar.activation(e8[q:q + 8, :], sct[q:q + 8, :], AF.Exp,
                                     bias=half_bias[q:q + 8, :], scale=0.5)
                e8_t[tC] = e8
            if doD:
                g, k = divmod(tD, GRP)
                gn = min(GRP, nt - g * GRP)
                if k == 0:
                    seP_g[g] = sep.tile([72, TILE], F32, tag="seP",
                                        name="seP")
                qD = 32 * k
                nc.tensor.matmul(seP_g[g][:], sumsel_sb[qD:qD + 8, k, :],
                                 e8_t[tD][qD:qD + 8, :], start=(k == 0),
                                 stop=(k == gn - 1))
                if k == gn - 1:
                    recf = rcs.tile([72, TILE], F32, tag="recf")
                    nc.vector.reciprocal(recf[:], seP_g.pop(g)[:])
                    recb = rcs.tile([72, TILE], BF16, tag="recb")
                    nc.scalar.activation(recb[:], recf[:], AF.Copy,
                                         bias=0.0, scale=1.0)
                    recb_g[g] = recb
            if doE:
                _emit_y(nc, lp, tE, L - 1, wb_t, proj_t, y_t, ys)
                del wb_t[tE]
                del w8_t[tE]
            if doF:
                ys_l = y_t.pop(tF)
                proj_t.pop(tF)
                agg = aggp.tile([128, TILE], F32, tag="agg")
                for l in range(L):
                    nc.tensor.matmul(agg[:], ident_sb[:], ys_l[l][:],
                                     start=(l == 0), stop=(l == L - 1))
                ot = outs.tile([128, TILE], BF16, tag="ot")
                nc.vector.tensor_copy(ot[:], agg[:])
                nc.sync.dma_start(out[:, ts(tF, TILE)], ot[:])


def _emit_y(nc, lp, t, l, wb_t, proj_t, y_t, ys):
    ALU = mybir.AluOpType
    wb = wb_t[t][l]
    y = ys.tile([128, TILE], mybir.dt.bfloat16, tag="y", name="y")
    if l == lp:
        # y = (wb + 1) * proj[lp]  (folds in the projlp term)
        nc.vector.scalar_tensor_tensor(
            y[:], wb[:], 1.0, proj_t[t][:, l, :], ALU.add, ALU.mult)
    else:
        nc.vector.tensor_mul(y[:], proj_t[t][:, l, :], wb[:])
    y_t[t].append(y)


def _build(lp: int, logis_b: float, nt: int = NT):
    nc = bacc.Bacc("TRN2", target_bir_lowering=False, debug=False,
                   num_devices=CORES)
    ins = {
        "xt": nc.dram_tensor("xt", [nt, 128, L, TILE], BF16,
                             kind="ExternalInput").ap(),
        "trans": nc.dram_tensor("trans", [128, 128], BF16,
                                kind="ExternalInput").ap(),
        "v8oh": nc.dram_tensor("v8oh", [128, GRP, L, 72], BF16,
                               kind="ExternalInput").ap(),
        "rowones": nc.dram_tensor("rowones", [72, L, 128], BF16,
                                  kind="ExternalInput").ap(),
        "sumsel": nc.dram_tensor("sumsel", [72, GRP, 72], BF16,
                                 kind="ExternalInput").ap(),
        "ident": nc.dram_tensor("ident", [128, 128], BF16,
                                kind="ExternalInput").ap(),
        "biasc": nc.dram_tensor("biasc", [128, 1], F32,
                                kind="ExternalInput").ap(),
    }
    out = nc.dram_tensor("out", [128, nt * TILE], BF16,
                         kind="ExternalOutput").ap()
    with tile.TileContext(nc) as tc:
        _body(tc, out, ins, lp, logis_b, nt)
    nc.compile()
    return nc


def _host_prep(inputs):
    nf = np.asarray(inputs["node_features"], np.float32)      # [L, N, F]
    trans = np.asarray(inputs["trans"], np.float32)           # [F, F]
    biasv = np.asarray(inputs["bias"], np.float32).reshape(F) # [F]
    theta = np.asarray(inputs["theta"], np.float32)           # [L, F, F]
    lw = np.asarray(inputs["logis_w"], np.float32).reshape(1, F)
    lb = float(np.asarray(inputs["logis_b"], np.float32).reshape(-1)[0])
    lp = int(np.asarray(inputs["layer_predict"]).reshape(-1)[0])

    v8 = theta @ lw[0]                                        # [L, F]
    v8oh = np.zeros((128, GRP, L, 72), np.float32)
    for k in range(GRP):
        for l in range(L):
            v8oh[:, k, l, 32 * k + l] = v8[l]
    rowones = np.zeros((72, L, 128), np.float32)
    for k in range(GRP):
        for l in range(L):
            rowones[32 * k + l, l, :] = 1.0
    sumsel = np.zeros((72, GRP, 72), np.float32)
    for k in range(GRP):
        sumsel[32 * k:32 * k + 8, k, 32 * k:32 * k + 8] = 1.0

    consts = {
        "trans": trans.astype(BF),
        "v8oh": v8oh.astype(BF),
        "rowones": rowones.astype(BF),
        "sumsel": sumsel.astype(BF),
        "ident": np.eye(128, dtype=np.float32).astype(BF),
        "biasc": np.ascontiguousarray(biasv.reshape(128, 1)),
    }

    # per-core packed node features: [NT, F, L, TILE] bf16
    nfb = nf.astype(BF)                                       # [L, N, F]

    def prep_core(c):
        sl = nfb[:, c * NS:(c + 1) * NS, :]                   # [L, NS, F]
        xt = np.transpose(sl, (2, 0, 1))                      # [F, L, NS]
        if NSP != NS:
            xt = np.concatenate(
                [xt, np.zeros((F, L, NSP - NS), BF)], axis=2)
        xt = np.ascontiguousarray(
            np.transpose(xt.reshape(F, L, NT, TILE), (2, 0, 1, 3)))
        return {"xt": xt, **consts}

    with ThreadPoolExecutor(CORES) as ex:
        in_maps = list(ex.map(prep_core, range(CORES)))
    return in_maps, lp, lb


_cache = {}


def _run(inputs, trace=False):
    from concourse.bass_utils import run_bass_kernel_spmd

    in_maps, lp, lb = _host_prep(inputs)
    key = (lp, round(lb, 8))
    if key not in _cache:
        _cache[key] = _build(lp, lb)
    nc = _cache[key]

    res = run_bass_kernel_spmd(nc, in_maps, core_ids=list(range(CORES)),
                               trace=trace)
    full = np.empty((N, F), np.float32)

    def fetch(c):
        o = np.asarray(res.results[c]["out"], dtype=np.float32)  # [128, NSP]
        full[c * NS:(c + 1) * NS] = o[:, :NS].T

    with ThreadPoolExecutor(CORES) as ex:
        list(ex.map(fetch, range(CORES)))
    return full, res


def kernel(**inputs) -> np.ndarray:
    out, _ = _run(inputs, trace=False)
    return out


# revision 23
# speedup vs baseline: 1.0970x; 1.0970x over previous
"""Trainium2 Bass kernel for nn_BitwiseMultipyLogis (gnn_message_passing).

Reference computation (L=8 layers, N=100000 nodes, F=128 features):
    proj    = tanh(node_features @ trans + bias)          # [L, N, F]
    bitwise = proj * proj[layer_predict]                  # [L, N, F]
    bitwise = einsum('lnf,lfg->lng', bitwise, theta)      # [L, N, F]
    scores  = sigmoid(bitwise @ logis_w[0] + logis_b)     # [L, N]
    weights = softmax(scores, axis=0)                     # [L, N]
    out     = proj[layer_predict] + sum_l weights[l]*proj[l]   # [N, F]

Algebraic simplification: theta only feeds the logis_w dot product, so
    scores[l,n] = sigmoid( sum_f proj[l,n,f]*proj[lp,n,f]*v[l,f] + logis_b )
with v[l] = theta[l] @ logis_w[0] precomputed on host.  This removes the
entire [L,N,F]x[L,F,F] einsum (half the FLOPs).

Device strategy (8 NeuronCores, data-parallel over N, 12500 nodes/core),
v3 — engine-balanced, instruction-count-minimized:
  * transposed layout [F=128 partitions, node columns]; host pre-packs
    node_features to [NT, 128, L, 512] bf16 so ONE DMA loads a whole tile.
  * projT = trans^T @ xT on TensorE (bf16), tanh+bias on ScalarE.
  * all 8 score rows accumulate into ONE [8, 512] PSUM tile via one-hot
    v columns; sigmoid+softmax with no table swap:
    exp(sigmoid(x)) = exp(0.5*tanh(x/2) + 0.5)  -> 2 activations per tile.
  * softmax denominators of a 4-tile group accumulate into one [97, 512]
    PSUM tile (rows 0/32/64/96) so ONE DVE reciprocal serves 4 tiles.
  * weights normalized as rows: w8 = e8 * bcast8(recip); the projlp term
    is folded into layer lp's y-multiply via scalar_tensor_tensor
    (y_lp = (wb_lp + 1) * proj_lp), so no final add is needed.
  * weight broadcast rows->128 partitions via [8,128] row-ones lhsT.
  * aggregation: 32 transpose-accumulate matmuls sum y tiles directly into
    a NODE-MAJOR bf16 PSUM tile [128, 4, 128]; ScalarE copies it to SBUF
    and 4 DMAs store [128 nodes, 128 f] blocks.  Output is node-major so
    the host does no transpose.
"""

import numpy as np
import ml_dtypes
from contextlib import ExitStack
from concurrent.futures import ThreadPoolExecutor

import concourse.bass as bass
import concourse.mybir as mybir
import concourse.tile as tile
from concourse import bacc
from concourse.bass import ts

BF16 = mybir.dt.bfloat16
F32 = mybir.dt.float32
AF = mybir.ActivationFunctionType
ALU = mybir.AluOpType

L, N, F = 8, 100000, 128
CORES = 8
NS = N // CORES            # 12500 nodes per core
TILE = 512                 # node columns per tile (one f32 PSUM bank)
NT = (NS + TILE - 1) // TILE   # 25
NSP = NT * TILE            # 12800 (padded)
GRP = 3                    # tiles per reciprocal group (rows 0/32/64)
NB = TILE // 128           # 128-node blocks per tile

BF = ml_dtypes.bfloat16


def _body(tc, out, ins, lp: int, logis_b: float, nt: int):
    nc = tc.nc
    with ExitStack() as ctx:
        const = ctx.enter_context(tc.tile_pool(name="const", bufs=1))
        xts = ctx.enter_context(tc.tile_pool(name="xts", bufs=7))
        projp = ctx.enter_context(tc.tile_pool(name="projp", bufs=3, space="PSUM"))
        projs = ctx.enter_context(tc.tile_pool(name="projs", bufs=9))
        bits = ctx.enter_context(tc.tile_pool(name="bits", bufs=18))
        scp = ctx.enter_context(tc.tile_pool(name="scp", bufs=1, space="PSUM"))
        scs = ctx.enter_context(tc.tile_pool(name="scs", bufs=2))
        e8s = ctx.enter_context(tc.tile_pool(name="e8s", bufs=8))
        sep = ctx.enter_context(tc.tile_pool(name="sep", bufs=1, space="PSUM"))
        rcs = ctx.enter_context(tc.tile_pool(name="rcs", bufs=3))
        w8s = ctx.enter_context(tc.tile_pool(name="w8s", bufs=3))
        wbp = ctx.enter_context(tc.tile_pool(name="wbp", bufs=2, space="PSUM"))
        ys = ctx.enter_context(tc.tile_pool(name="ys", bufs=18))
        aggp = ctx.enter_context(tc.tile_pool(name="aggp", bufs=1, space="PSUM"))
        outs = ctx.enter_context(tc.tile_pool(name="outs", bufs=2))

        trans_sb = const.tile([128, 128], BF16)
        nc.sync.dma_start(trans_sb[:], ins["trans"])
        # v8oh[:, k, l, :]: one-hot lhsT; column 32k+l holds v[l] so the
        # score row of layer l for group-slot k lands at PSUM partition 32k+l.
        v8oh_sb = const.tile([128, GRP, L, 72], BF16)
        nc.sync.dma_start(v8oh_sb[:], ins["v8oh"])
        # rowones[32k+l, l, :] all-ones: broadcasts weight row 32k+l across
        # all 128 output partitions (lhsT base partition 32k is legal).
        rowones_sb = const.tile([72, L, 128], BF16)
        nc.sync.dma_start(rowones_sb[:], ins["rowones"])
        # sumsel[32k:32k+8, k, :]: ones in columns 32k..32k+7 -> the layer
        # sum lands REPLICATED on partitions 32k..32k+7, so the normalize
        # multiply can read it without a separate broadcast matmul.
        sumsel_sb = const.tile([72, GRP, 72], BF16)
        nc.sync.dma_start(sumsel_sb[:], ins["sumsel"])
        ident_sb = const.tile([128, 128], BF16)
        nc.sync.dma_start(ident_sb[:], ins["ident"])
        bias_sb = const.tile([128, 1], F32)
        nc.sync.dma_start(bias_sb[:], ins["biasc"])
        lb_bias = const.tile([128, 1], F32)
        nc.gpsimd.memset(lb_bias[:], 0.5 * logis_b)
        half_bias = const.tile([128, 1], F32)
        nc.gpsimd.memset(half_bias[:], 0.5)
        zbias = const.tile([128, 1], F32)
        nc.gpsimd.memset(zbias[:], 0.0)

        xt = ins["xt"]
        xt_tiles = {}
        PF = 5  # DMA prefetch distance (tiles)

        def issue_in_dma(t):
            if t < nt:
                xt_sb = xts.tile([128, L, TILE], BF16, tag="xt")
                nc.sync.dma_start(xt_sb[:], xt[t])
                xt_tiles[t] = xt_sb

        proj_t, bits_t, sc_t, e8_t, y_t, wb_t = {}, {}, {}, {}, {}, {}
        seP_g, recb_g, w8_t = {}, {}, {}

        # Deep software pipeline with instruction-level interleave: iteration
        # i processes tile i's proj, tile i-1's bits, tile i-2's scores,
        # tile i-3's denominator, tile i-6's broadcasts/products and tile
        # i-7's aggregation.  Interleaving the matmul families keeps every
        # PE instruction's dependencies satisfied ~an iteration in advance,
        # so the PE streams at its back-to-back pitch instead of stalling
        # on fresh cross-engine semaphores.

        for t in range(min(PF, nt)):
            issue_in_dma(t)
        for i in range(nt + 8):
            issue_in_dma(i + PF)
            tA, tB, tC, tD, tE, tF = i, i - 1, i - 2, i - 3, i - 6, i - 7
            doA = tA < nt
            doB = 0 <= tB < nt
            doC = 0 <= tC < nt
            doD = 0 <= tD < nt
            doE = 0 <= tE < nt
            doF = 0 <= tF < nt

            if doA:
                proj = projs.tile([128, L, TILE], BF16, tag="proj")
                proj_t[tA] = proj
            if doB:
                bits_t[tB] = []
            if doC:
                sc_t[tC] = scp.tile([72, TILE], F32, tag="sc", name="sc")
            if doE:
                # normalized weight rows for tile tE (all SBUF bf16: 2x DVE)
                g, k = divmod(tE, GRP)
                q = 32 * k
                w8 = w8s.tile([72, TILE], BF16, tag="w8")
                nc.vector.tensor_mul(w8[q:q + 8, :], e8_t.pop(tE)[q:q + 8, :],
                                     recb_g[g][q:q + 8, :])
                w8_t[tE] = w8
                y_t[tE] = []
                wb_t[tE] = []

            for l in range(L):
                if doA:
                    pp = projp.tile([128, TILE], F32, tag="pp")
                    nc.tensor.matmul(pp[:], trans_sb[:],
                                     xt_tiles[tA][:, l, :],
                                     start=True, stop=True)
                    nc.scalar.activation(proj_t[tA][:, l, :], pp[:], AF.Tanh,
                                         bias=bias_sb[:, 0:1], scale=1.0)
                if doB:
                    bit = bits.tile([128, TILE], BF16, tag="bit")
                    eng = nc.vector if l in (0, 2, 4, 6) else nc.gpsimd
                    eng.tensor_mul(bit[:], proj_t[tB][:, l, :],
                                   proj_t[tB][:, lp, :])
                    bits_t[tB].append(bit)
                if doC:
                    kC = tC % GRP
                    nc.tensor.matmul(sc_t[tC][0:32 * kC + 8, :],
                                     v8oh_sb[:, kC, l, 0:32 * kC + 8],
                                     bits_t[tC][l][:],
                                     start=(l == 0), stop=(l == L - 1))
                if doE:
                    qE = 32 * (tE % GRP)
                    wb = wbp.tile([128, TILE], F32, tag="wb")
                    nc.tensor.matmul(wb[:], rowones_sb[qE:qE + 8, l, :],
                                     w8_t[tE][qE:qE + 8, :],
                                     start=True, stop=True)
                    wb_t[tE].append(wb)
                    if l >= 1:
                        _emit_y(nc, lp, tE, l - 1, wb_t, proj_t, y_t, ys)

            if doA:
                xt_tiles.pop(tA)
            if doC:
                del bits_t[tC]
                sc = sc_t.pop(tC)
                # e = exp(sigmoid(raw + lb)) with no table swap:
                # u = tanh(0.5*raw + 0.5*lb); e = exp(0.5*u + 0.5)
                kC = tC % GRP
                q = 32 * kC
                sct = scs.tile([72, TILE], F32, tag="sct")
                nc.scalar.activation(sct[q:q + 8, :], sc[q:q + 8, :], AF.Tanh,
                                     bias=lb_bias[q:q + 8, :], scale=0.5)
                e8 = e8s.tile([72, TILE], BF16, tag="e8")
                nc.scalar.activation(e8[q:q + 8, :], sct[q:q + 8, :], AF.Exp,
                                     bias=half_bias[q:q + 8, :], scale=0.5)
                e8_t[tC] = e8
            if doD:
                g, k = divmod(tD, GRP)
                gn = min(GRP, nt - g * GRP)
                if k == 0:
                    seP_g[g] = sep.tile([72, TILE], F32, tag="seP",
                                        name="seP")
                qD = 32 * k
                nc.tensor.matmul(seP_g[g][:], sumsel_sb[qD:qD + 8, k, :],
                                 e8_t[tD][qD:qD + 8, :], start=(k == 0),
                                 stop=(k == gn - 1))
                if k == gn - 1:
                    recf = rcs.tile([72, TILE], F32, tag="recf")
                    nc.vector.reciprocal(recf[:], seP_g.pop(g)[:])
                    recb = rcs.tile([72, TILE], BF16, tag="recb")
                    nc.scalar.activation(recb[:], recf[:], AF.Copy,
                                         bias=0.0, scale=1.0)
                    recb_g[g] = recb
            if doE:
                _emit_y(nc, lp, tE, L - 1, wb_t, proj_t, y_t, ys)
                del wb_t[tE]
                del w8_t[tE]
            if doF:
                ys_l = y_t.pop(tF)
                proj_t.pop(tF)
                agg = aggp.tile([128, TILE], F32, tag="agg")
                for l in range(L):
                    nc.tensor.matmul(agg[:], ident_sb[:], ys_l[l][:],
                                     start=(l == 0), stop=(l == L - 1))
                ot = outs.tile([128, TILE], BF16, tag="ot")
                nc.vector.tensor_copy(ot[:], agg[:])
                nc.sync.dma_start(out[:, ts(tF, TILE)], ot[:])


def _emit_y(nc, lp, t, l, wb_t, proj_t, y_t, ys):
    ALU = mybir.AluOpType
    wb = wb_t[t][l]
    y = ys.tile([128, TILE], mybir.dt.bfloat16, tag="y", name="y")
    if l == lp:
        # y = (wb + 1) * proj[lp]  (folds in the projlp term)
        nc.vector.scalar_tensor_tensor(
            y[:], wb[:], 1.0, proj_t[t][:, l, :], ALU.add, ALU.mult)
    else:
        nc.vector.tensor_mul(y[:], proj_t[t][:, l, :], wb[:])
    y_t[t].append(y)


def _build(lp: int, logis_b: float, nt: int = NT):
    nc = bacc.Bacc("TRN2", target_bir_lowering=False, debug=False,
                   num_devices=CORES)
    ins = {
        "xt": nc.dram_tensor("xt", [nt, 128, L, TILE], BF16,
                             kind="ExternalInput").ap(),
        "trans": nc.dram_tensor("trans", [128, 128], BF16,
                                kind="ExternalInput").ap(),
        "v8oh": nc.dram_tensor("v8oh", [128, GRP, L, 72], BF16,
                               kind="ExternalInput").ap(),
        "rowones": nc.dram_tensor("rowones", [72, L, 128], BF16,
                                  kind="ExternalInput").ap(),
        "sumsel": nc.dram_tensor("sumsel", [72, GRP, 72], BF16,
                                 kind="ExternalInput").ap(),
        "ident": nc.dram_tensor("ident", [128, 128], BF16,
                                kind="ExternalInput").ap(),
        "biasc": nc.dram_tensor("biasc", [128, 1], F32,
                                kind="ExternalInput").ap(),
    }
    out = nc.dram_tensor("out", [128, nt * TILE], BF16,
                         kind="ExternalOutput").ap()
    with tile.TileContext(nc) as tc:
        _body(tc, out, ins, lp, logis_b, nt)
    nc.compile()
    return nc


def _host_prep(inputs):
    nf = np.asarray(inputs["node_features"], np.float32)      # [L, N, F]
    trans = np.asarray(inputs["trans"], np.float32)           # [F, F]
    biasv = np.asarray(inputs["bias"], np.float32).reshape(F) # [F]
    theta = np.asarray(inputs["theta"], np.float32)           # [L, F, F]
    lw = np.asarray(inputs["logis_w"], np.float32).reshape(1, F)
    lb = float(np.asarray(inputs["logis_b"], np.float32).reshape(-1)[0])
    lp = int(np.asarray(inputs["layer_predict"]).reshape(-1)[0])

    v8 = theta @ lw[0]                                        # [L, F]
    v8oh = np.zeros((128, GRP, L, 72), np.float32)
    for k in range(GRP):
        for l in range(L):
            v8oh[:, k, l, 32 * k + l] = v8[l]
    rowones = np.zeros((72, L, 128), np.float32)
    for k in range(GRP):
        for l in range(L):
            rowones[32 * k + l, l, :] = 1.0
    sumsel = np.zeros((72, GRP, 72), np.float32)
    for k in range(GRP):
        sumsel[32 * k:32 * k + 8, k, 32 * k:32 * k + 8] = 1.0

    consts = {
        "trans": trans.astype(BF),
        "v8oh": v8oh.astype(BF),
        "rowones": rowones.astype(BF),
        "sumsel": sumsel.astype(BF),
        "ident": np.eye(128, dtype=np.float32).astype(BF),
        "biasc": np.ascontiguousarray(biasv.reshape(128, 1)),
    }

    # per-core packed node features: [NT, F, L, TILE] bf16
    nfb = nf.astype(BF)                                       # [L, N, F]

    def prep_core(c):
        sl = nfb[:, c * NS:(c + 1) * NS, :]                   # [L, NS, F]
        xt = np.transpose(sl, (2, 0, 1))                      # [F, L, NS]
        if NSP != NS:
            xt = np.concatenate(
                [xt, np.zeros((F, L, NSP - NS), BF)], axis=2)
        xt = np.ascontiguousarray(
            np.transpose(xt.reshape(F, L, NT, TILE), (2, 0, 1, 3)))
        return {"xt": xt, **consts}

    with ThreadPoolExecutor(CORES) as ex:
        in_maps = list(ex.map(prep_core, range(CORES)))
    return in_maps, lp, lb


_cache = {}


def _run(inputs, trace=False):
    from concourse.bass_utils import run_bass_kernel_spmd

    in_maps, lp, lb = _host_prep(inputs)
    key = (lp, round(lb, 8))
    if key not in _cache:
        _cache[key] = _build(lp, lb)
    nc = _cache[key]

    res = run_bass_kernel_spmd(nc, in_maps, core_ids=list(range(CORES)),
                               trace=trace)
    full = np.empty((N, F), np.float32)

    def fetch(c):
        o = np.asarray(res.results[c]["out"], dtype=np.float32)  # [128, NSP]
        full[c * NS:(c + 1) * NS] = o[:, :NS].T

    with ThreadPoolExecutor(CORES) as ex:
        list(ex.map(fetch, range(CORES)))
    return full, res


def kernel(**inputs) -> np.ndarray:
    out, _ = _run(inputs, trace=False)
    return out
